# revision 61
# baseline (speedup 1.0000x reference)
import sys

sys.path.insert(0, "/opt/trn_rl_repo")
import numpy as np

import concourse.mybir as mybir
import concourse.tile as tile
from concourse import bacc
from concourse.bass_utils import run_bass_kernel_spmd

F32 = mybir.dt.float32
F32R = mybir.dt.float32r
AF = mybir.ActivationFunctionType
ALU = mybir.AluOpType
AX = mybir.AxisListType

B, M, NPTS, D, FD, S = 4, 8, 4096, 256, 512, 128
NT = 16          # 128-point tiles per core
HALF = NT * 128  # 2048 points per core
EPS_LN = 1e-5
RG = [[0, 1], [2, 3], [4, 5], [6, 7]]

TRACE = False          # test.py may set kernel.TRACE = True
LAST_RESULT = None     # exposes BassKernelResults to test.py

_PROGRAM_CACHE = {}


def _build_program(flags):
    (has_pb, has_vb, has_a1b, has_fb, has_pa, has_va, has_aa, a2bias) = flags
    nc = bacc.Bacc("TRN2", target_bir_lowering=False, debug=False)

    pft_d = nc.dram_tensor("pft", [NT, 128, 256], F32R, kind="ExternalInput")
    vft_d = nc.dram_tensor("vft", [NT, M, 128, 256], F32R, kind="ExternalInput")
    vfn_d = nc.dram_tensor("vfn", [NT, M, 128, 256], F32, kind="ExternalInput")
    spf_d = nc.dram_tensor("spf", [128, NT], F32, kind="ExternalInput")
    pw_d = nc.dram_tensor("pw", [128, 2 * 258], F32R, kind="ExternalInput")
    vw_d = nc.dram_tensor("vw", [128, 2 * 258], F32R, kind="ExternalInput")
    a1w_d = nc.dram_tensor("a1w", [128, 4 * 258], F32R, kind="ExternalInput")
    fw_d = nc.dram_tensor("fw", [128, 4 * 514], F32R, kind="ExternalInput")
    a2b_d = nc.dram_tensor("a2b", [128, 256], F32, kind="ExternalInput")
    iota_d = nc.dram_tensor("iota", [128, 128], F32, kind="ExternalInput")
    eyer_d = nc.dram_tensor("eyer", [128, 128], F32R, kind="ExternalInput")
    outp_d = nc.dram_tensor("outp", [NT, 128, 512], F32, kind="ExternalOutput")

    need_bias = has_pb or has_vb or has_a1b or has_fb
    if need_bias:
        ones1_d = nc.dram_tensor("ones1", [1, 128], F32R, kind="ExternalInput")
    if has_pb:
        pbr_d = nc.dram_tensor("pbr", [1, 258], F32R, kind="ExternalInput")
    if has_vb:
        vbr_d = nc.dram_tensor("vbr", [1, 258], F32R, kind="ExternalInput")
    if has_a1b:
        a1br_d = nc.dram_tensor("a1br", [1, 258], F32R, kind="ExternalInput")
    if has_fb:
        fbr_d = nc.dram_tensor("fbr", [1, 514], F32R, kind="ExternalInput")
    # interior gain/bias broadcast images (gain row, then be row), [128,512]
    if has_pa:
        paff_d = nc.dram_tensor("paff", [128, 512], F32, kind="ExternalInput")
    if has_va:
        vaff_d = nc.dram_tensor("vaff", [128, 512], F32, kind="ExternalInput")
    if has_aa:
        aaff_d = nc.dram_tensor("aaff", [128, 512], F32, kind="ExternalInput")

    with tile.TileContext(nc) as tc:
        with tc.tile_pool(name="wp", bufs=1) as wp, \
             tc.tile_pool(name="pers", bufs=1) as pers, \
             tc.tile_pool(name="dp", bufs=1, space="DRAM") as dp:
            # ---- persistent weights/constants ----
            pw_sb = wp.tile([128, 2 * 258], F32R, tag="pw")
            vw_sb = wp.tile([128, 2 * 258], F32R, tag="vw")
            a1w_sb = wp.tile([128, 4 * 258], F32R, tag="a1w")
            fw_sb = wp.tile([128, 4 * 514], F32R, tag="fw")
            a2b_sb = wp.tile([128, 256], F32, tag="a2b")
            iota_sb = wp.tile([128, 128], F32, tag="iota")
            eyer_sb = wp.tile([128, 128], F32R, tag="eyer")
            spf_sb = wp.tile([128, NT], F32, tag="spf")
            for sb, d in ((pw_sb, pw_d), (vw_sb, vw_d), (a1w_sb, a1w_d),
                          (fw_sb, fw_d), (a2b_sb, a2b_d), (iota_sb, iota_d),
                          (eyer_sb, eyer_d), (spf_sb, spf_d)):
                nc.gpsimd.dma_start(sb[:], d[:])
            if need_bias:
                ones1_sb = wp.tile([1, 128], F32R, tag="ones1")
                nc.gpsimd.dma_start(ones1_sb[:], ones1_d[:])
            if has_pb:
                pbr_sb = wp.tile([1, 258], F32R, tag="pbr")
                nc.gpsimd.dma_start(pbr_sb[:], pbr_d[:])
            if has_vb:
                vbr_sb = wp.tile([1, 258], F32R, tag="vbr")
                nc.gpsimd.dma_start(vbr_sb[:], vbr_d[:])
            if has_a1b:
                a1br_sb = wp.tile([1, 258], F32R, tag="a1br")
                nc.gpsimd.dma_start(a1br_sb[:], a1br_d[:])
            if has_fb:
                fbr_sb = wp.tile([1, 514], F32R, tag="fbr")
                nc.gpsimd.dma_start(fbr_sb[:], fbr_d[:])
            aff_sb = {}
            for key, flag, d in (("p", has_pa, "paff"), ("v", has_va, "vaff"),
                                 ("a", has_aa, "aaff")):
                if flag:
                    t_ = wp.tile([128, 512], F32, tag=d)
                    d_t = {"paff": paff_d, "vaff": vaff_d, "aaff": aaff_d}[d]
                    nc.gpsimd.dma_start(t_[:], d_t[:])
                    aff_sb[key] = t_

            # ---- persistent state ----
            pp_sb = [pers.tile([128, 266], F32R, tag=f"pp{t}", name=f"pp{t}") for t in range(NT)]
            ppT_sb = [pers.tile([128, 256], F32R, tag=f"ppT{t}", name=f"ppT{t}") for t in range(NT)]
            oh_sb = [pers.tile([128, 128], F32R, tag=f"oh{t}", name=f"oh{t}") for t in range(NT)]
            ohT_sb = [pers.tile([128, 128], F32R, tag=f"ohT{t}", name=f"ohT{t}") for t in range(NT)]
            nf2_sb = pers.tile([128, NT], F32, tag="nf2", name="nf2")
            apre_sb = pers.tile([128, 128], F32, tag="apre", name="apre")
            am_sb = pers.tile([128, 128], F32, tag="amall", name="amall")
            dots_sb = pers.tile([128, NT], F32, tag="dots", name="dots")
            prods_sb = pers.tile([128, NT], F32, tag="prods", name="prods")
            sims_sb = pers.tile([128, NT], F32, tag="sims", name="sims")
            ref_sb = pers.tile([128, 128], F32, tag="refined", name="refined")
            na_sb = pers.tile([128, 128], F32, tag="naall", name="naall")
            nmax_sb = pers.tile([128, NT], F32, tag="nmax", name="nmax")
            sume_sb = pers.tile([128, NT], F32, tag="sume", name="sume")
            rse_sb = pers.tile([128, NT], F32, tag="rse", name="rse")
            segp_sb = pers.tile([128, 266], F32, tag="segpre", name="segpre")
            seg_sb = pers.tile([128, 266], F32, tag="segsb", name="segsb")
            mean_sb = pers.tile([128, 266], F32R, tag="meansb", name="meansb")
            eps_sb = pers.tile([128, 1], F32, tag="epsln", name="epsln")
            nc.gpsimd.memset(eps_sb[:], EPS_LN)
            a2bv_sb = pers.tile([128, 1], F32, tag="a2bv", name="a2bv")
            nc.gpsimd.memset(a2bv_sb[:], a2bias)
            onecol_sb = pers.tile([128, 2], F32, tag="onecol", name="onecol")
            nc.gpsimd.memset(onecol_sb[:], 1.0)

            bounce_in = dp.tile([128, 266], F32, tag="bin")
            bounce_out = dp.tile([128, 266], F32, tag="bout")

            def asf32(ap):
                return ap.bitcast(F32) if ap.dtype == F32R else ap

            def mm_group(out_ap, lhs_list, rhs_list, bias_pair=None):
                n = len(lhs_list) + (1 if bias_pair is not None else 0)
                i = 0
                for l, r in zip(lhs_list, rhs_list):
                    nc.tensor.matmul(out_ap, l, r,
                                     start=(i == 0), stop=(i == n - 1))
                    i += 1
                if bias_pair is not None:
                    nc.tensor.matmul(out_ap, bias_pair[0], bias_pair[1],
                                     start=(i == 0), stop=True)

            def ln_stats(pool, ps, width, tagp):
                # ps: psum [128, width+1] with mean-sum col at `width`.
                # returns (r, negmr) [128,1] APs
                sq = pool.tile([128, width], F32, tag=tagp + "sq")
                s2 = pool.tile([128, 1], F32, tag=tagp + "s2")
                nc.scalar.activation(out=sq[:], in_=ps[:, 0:width],
                                     func=AF.Square, accum_out=s2[:])
                m = pool.tile([128, 1], F32, tag=tagp + "m")
                nc.vector.tensor_scalar(out=m[:], in0=ps[:, width:width + 1],
                                        scalar1=1.0 / width, scalar2=None,
                                        op0=ALU.mult)
                msq = pool.tile([128, 1], F32, tag=tagp + "msq")
                nc.gpsimd.tensor_tensor(out=msq[:], in0=m[:], in1=m[:], op=ALU.mult)
                var = pool.tile([128, 1], F32, tag=tagp + "var")
                nc.vector.scalar_tensor_tensor(out=var[:], in0=s2[:],
                                               scalar=1.0 / width, in1=msq[:],
                                               op0=ALU.mult, op1=ALU.subtract)
                std = pool.tile([128, 1], F32, tag=tagp + "std")
                nc.scalar.activation(out=std[:], in_=var[:], func=AF.Sqrt,
                                     bias=eps_sb[:])
                r = pool.tile([128, 1], F32, tag=tagp + "r")
                nc.vector.reciprocal(r[:], std[:])
                negmr = pool.tile([128, 1], F32, tag=tagp + "negmr")
                nc.vector.scalar_tensor_tensor(out=negmr[:], in0=m[:], scalar=-1.0,
                                               in1=r[:], op0=ALU.mult, op1=ALU.mult)
                return r, negmr

            def ln_apply(pool, out_ap, ps_ap, r, negmr, aff_key, relu, tagp):
                # out = [relu]( LN(x)*g + be ); unit-affine fast path uses one ACT op
                if aff_key is None:
                    nc.scalar.activation(out=out_ap, in_=ps_ap,
                                         func=(AF.Relu if relu else AF.Identity),
                                         scale=r[:], bias=negmr[:])
                    return
                g_ap = aff_sb[aff_key][:, 0:256]
                be_ap = aff_sb[aff_key][:, 256:512]
                t1 = pool.tile([128, 256], F32, tag=tagp + "t1")
                nc.scalar.activation(out=t1[:], in_=ps_ap, func=AF.Identity,
                                     scale=r[:], bias=negmr[:])
                t2 = pool.tile([128, 256], F32, tag=tagp + "t2")
                nc.vector.tensor_tensor(out=t2[:], in0=t1[:], in1=g_ap, op=ALU.mult)
                if relu:
                    t3 = pool.tile([128, 256], F32, tag=tagp + "t3")
                    nc.vector.scalar_tensor_tensor(out=t3[:], in0=t2[:], scalar=0.0,
                                                   in1=be_ap, op0=ALU.bypass,
                                                   op1=ALU.add)
                    nc.vector.tensor_scalar(out=out_ap, in0=t3[:], scalar1=0.0,
                                            scalar2=None, op0=ALU.max)
                else:
                    nc.vector.scalar_tensor_tensor(out=out_ap, in0=t2[:], scalar=0.0,
                                                   in1=be_ap, op0=ALU.bypass,
                                                   op1=ALU.add)

            def transpose256(pool_ps, pool_sb, src_sb_ap0, src_sb_ap1, dst_sb_ap,
                             tagp):
                tp = pool_ps.tile([128, 256], F32R, tag=tagp)
                nc.tensor.transpose(tp[:, 0:128], src_sb_ap0, eyer_sb[:])
                nc.tensor.transpose(tp[:, 128:256], src_sb_ap1, eyer_sb[:])
                nc.scalar.activation(out=dst_sb_ap, in_=asf32(tp[:]),
                                     func=AF.Copy)

            # ================= phase 1 =================
            with tc.tile_pool(name="s1", bufs=2) as s1, \
                 tc.tile_pool(name="p1", bufs=2, space="PSUM") as p1, \
                 tc.tile_pool(name="pg", bufs=1, space="PSUM") as pg:
                for t in range(NT):
                    pft_t = s1.tile([128, 256], F32R, tag="pft")
                    nc.gpsimd.dma_start(pft_t[:], pft_d[t])
                    pp_ps = p1.tile([128, 258], F32, tag="mm")
                    mm_group(pp_ps[:], [pft_t[:, 0:128], pft_t[:, 128:256]],
                             [pw_sb[:, 0:258], pw_sb[:, 258:516]],
                             (ones1_sb[:], pbr_sb[:]) if has_pb else None)
                    r0, nm0 = ln_stats(s1, pp_ps, 256, "pp")
                    ln_apply(s1, pp_sb[t][:, 0:256], pp_ps[:, 0:256], r0, nm0,
                             "p" if has_pa else None, True, "pp")
                    # |f|^2 per point
                    sqn = s1.tile([128, 256], F32, tag="sqn")
                    nc.vector.scalar_tensor_tensor(
                        out=sqn[:], in0=asf32(pp_sb[t][:, 0:256]), scalar=0.0,
                        in1=asf32(pp_sb[t][:, 0:256]), op0=ALU.bypass,
                        op1=ALU.mult, accum_out=nf2_sb[:, t:t + 1])
                    # one-hot segment matrix
                    nc.vector.tensor_scalar(out=oh_sb[t][:], in0=iota_sb[:],
                                            scalar1=spf_sb[:, t:t + 1],
                                            scalar2=None, op0=ALU.is_equal)
                    nc.scalar.activation(out=pp_sb[t][:, 264:266],
                                         in_=onecol_sb[:], func=AF.Copy)
                    transpose256(p1, s1, pp_sb[t][:, 0:128], pp_sb[t][:, 128:256],
                                 ppT_sb[t][:], "tp")

                    for v in range(M):
                        vft_tv = s1.tile([128, 256], F32R, tag="vft", bufs=3)
                        nc.gpsimd.dma_start(vft_tv[:], vft_d[t, v])
                        vp_ps = p1.tile([128, 258], F32, tag="mm")
                        mm_group(vp_ps[:], [vft_tv[:, 0:128], vft_tv[:, 128:256]],
                                 [vw_sb[:, 0:258], vw_sb[:, 258:516]],
                                 (ones1_sb[:], vbr_sb[:]) if has_vb else None)
                        r1, nm1 = ln_stats(s1, vp_ps, 256, "vp")
                        vp_t = s1.tile([128, 256], F32R, tag="vpsb")
                        ln_apply(s1, vp_t[:], vp_ps[:, 0:256], r1, nm1,
                                 "v" if has_va else None, True, "vp")
                        vpT_t = s1.tile([128, 256], F32R, tag="vpT")
                        transpose256(p1, s1, vp_t[:, 0:128], vp_t[:, 128:256],
                                     vpT_t[:], "tp")
                        h_ps = p1.tile([128, 258], F32, tag="hmm")
                        mm_group(h_ps[:],
                                 [ppT_sb[t][:, 0:128], ppT_sb[t][:, 128:256],
                                  vpT_t[:, 0:128], vpT_t[:, 128:256]],
                                 [a1w_sb[:, 0:258], a1w_sb[:, 258:516],
                                  a1w_sb[:, 516:774], a1w_sb[:, 774:1032]],
                                 (ones1_sb[:], a1br_sb[:]) if has_a1b else None)
                        r2, nm2 = ln_stats(s1, h_ps, 256, "h")
                        h_t = s1.tile([128, 256], F32, tag="hsb")
                        ln_apply(s1, h_t[:], h_ps[:, 0:256], r2, nm2,
                                 "a" if has_aa else None, True, "h")
                        sqd = s1.tile([128, 256], F32, tag="sqd")
                        nc.vector.scalar_tensor_tensor(
                            out=sqd[:], in0=h_t[:], scalar=0.0, in1=a2b_sb[:],
                            op0=ALU.bypass, op1=ALU.mult,
                            accum_out=apre_sb[:, 8 * t + v:8 * t + v + 1])

                # ---- phase 1b: sigmoid (one ACT table switch) ----
                for t in range(NT):
                    nc.scalar.activation(out=pp_sb[t][:, 256:264],
                                         in_=apre_sb[:, 8 * t:8 * t + 8],
                                         func=AF.Sigmoid, bias=a2bv_sb[:])

                # ---- phase 1c: segment sums + AllReduce ----
                seg_ps = pg.tile([128, 266], F32, tag="seg")
                for t in range(NT):
                    nc.tensor.matmul(seg_ps[:], oh_sb[t][:], pp_sb[t][:, 0:266],
                                     start=(t == 0), stop=(t == NT - 1))
                nc.scalar.activation(out=segp_sb[:], in_=seg_ps[:], func=AF.Copy)
                nc.gpsimd.dma_start(bounce_in[:], segp_sb[:])
                nc.gpsimd.collective_compute(
                    "AllReduce", ALU.add, replica_groups=RG,
                    ins=[bounce_in.opt()], outs=[bounce_out.opt()])
                nc.gpsimd.dma_start(seg_sb[:], bounce_out[:])

            # ================= phase 2 =================
            with tc.tile_pool(name="s2", bufs=2) as s2, \
                 tc.tile_pool(name="p2t", bufs=2, space="PSUM") as p2t, \
                 tc.tile_pool(name="p2g", bufs=2, space="PSUM") as p2g, \
                 tc.tile_pool(name="p2m", bufs=1, space="PSUM") as p2m:
                # segment means + |fm|^2
                cntc = s2.tile([128, 1], F32, tag="cntc")
                nc.vector.tensor_scalar(out=cntc[:], in0=seg_sb[:, 264:265],
                                        scalar1=1.0, scalar2=None, op0=ALU.max)
                rc = s2.tile([128, 1], F32, tag="rc")
                nc.vector.reciprocal(rc[:], cntc[:])
                nc.vector.tensor_scalar(out=mean_sb[:, 0:264], in0=seg_sb[:, 0:264],
                                        scalar1=rc[:], scalar2=None, op0=ALU.mult)
                sqm = s2.tile([128, 256], F32, tag="sqm")
                m2tmp = s2.tile([128, 1], F32, tag="m2tmp")
                nc.vector.scalar_tensor_tensor(
                    out=sqm[:], in0=asf32(mean_sb[:, 0:256]), scalar=0.0,
                    in1=asf32(mean_sb[:, 0:256]), op0=ALU.bypass, op1=ALU.mult,
                    accum_out=m2tmp[:])
                nc.scalar.activation(out=mean_sb[:, 264:265], in_=m2tmp[:],
                                     func=AF.Copy)
                nc.scalar.activation(out=mean_sb[:, 265:266], in_=m2tmp[:],
                                     func=AF.Copy)

                # ---- phase 2a: gather + cosine ----
                for t in range(NT):
                    tpo = p2t.tile([128, 256], F32R, tag="tp2")
                    nc.tensor.transpose(tpo[:, 0:128], oh_sb[t][:], eyer_sb[:])
                    nc.scalar.activation(out=ohT_sb[t][:],
                                         in_=asf32(tpo[:, 0:128]), func=AF.Copy)
                    gath = p2g.tile([128, 266], F32, tag="gath")
                    nc.tensor.matmul(gath[:], ohT_sb[t][:], mean_sb[:, 0:266],
                                     start=True, stop=True)
                    sqg = s2.tile([128, 256], F32, tag="sqg")
                    nc.vector.scalar_tensor_tensor(
                        out=sqg[:], in0=asf32(pp_sb[t][:, 0:256]), scalar=0.0,
                        in1=gath[:, 0:256], op0=ALU.bypass, op1=ALU.mult,
                        accum_out=dots_sb[:, t:t + 1])
                    nc.vector.tensor_tensor(out=prods_sb[:, t:t + 1],
                                            in0=nf2_sb[:, t:t + 1],
                                            in1=gath[:, 264:265], op=ALU.mult)
                    nc.scalar.activation(out=am_sb[:, 8 * t:8 * t + 8],
                                         in_=gath[:, 256:264], func=AF.Copy)

                dens = s2.tile([128, NT], F32, tag="dens")
                nc.scalar.activation(out=dens[:], in_=prods_sb[:], func=AF.Sqrt)
                densc = s2.tile([128, NT], F32, tag="densc")
                nc.vector.tensor_scalar(out=densc[:], in0=dens[:], scalar1=1e-16,
                                        scalar2=None, op0=ALU.max)
                rden = s2.tile([128, NT], F32, tag="rden")
                nc.vector.reciprocal(rden[:], densc[:])
                nc.vector.tensor_tensor(out=sims_sb[:], in0=dots_sb[:], in1=rden[:],
                                        op=ALU.mult)
                for t in range(NT):
                    dtl = s2.tile([128, 8], F32, tag="dtl")
                    nc.gpsimd.tensor_tensor(out=dtl[:],
                                            in0=asf32(pp_sb[t][:, 256:264]),
                                            in1=am_sb[:, 8 * t:8 * t + 8],
                                            op=ALU.subtract)
                    nc.vector.scalar_tensor_tensor(
                        out=ref_sb[:, 8 * t:8 * t + 8], in0=dtl[:],
                        scalar=sims_sb[:, t:t + 1],
                        in1=am_sb[:, 8 * t:8 * t + 8], op0=ALU.mult, op1=ALU.add)

                # ---- phase 2b: softmax over views (exp table switch) ----
                for t in range(NT):
                    nc.vector.tensor_reduce(out=nmax_sb[:, t:t + 1],
                                            in_=ref_sb[:, 8 * t:8 * t + 8],
                                            axis=AX.X, op=ALU.max, negate=True)
                for t in range(NT):
                    nc.scalar.activation(out=na_sb[:, 8 * t:8 * t + 8],
                                         in_=ref_sb[:, 8 * t:8 * t + 8],
                                         func=AF.Exp,
                                         bias=nmax_sb[:, t:t + 1],
                                         accum_out=sume_sb[:, t:t + 1])
                nc.vector.reciprocal(rse_sb[:], sume_sb[:])
                for t in range(NT):
                    nc.vector.tensor_scalar(out=na_sb[:, 8 * t:8 * t + 8],
                                            in0=na_sb[:, 8 * t:8 * t + 8],
                                            scalar1=rse_sb[:, t:t + 1],
                                            scalar2=None, op0=ALU.mult)

                # ---- phase 2c: weighted sum, wvp, final ----
                for t in range(NT):
                    w_prev = None
                    for v in range(M):
                        vfn_tv = s2.tile([128, 256], F32, tag="vfn", bufs=4)
                        nc.gpsimd.dma_start(vfn_tv[:], vfn_d[t, v])
                        na_col = na_sb[:, 8 * t + v:8 * t + v + 1]
                        w = s2.tile([128, 256], F32R, tag="wacc", bufs=2)
                        if v == 0:
                            nc.vector.tensor_scalar(out=w[:], in0=vfn_tv[:],
                                                    scalar1=na_col, scalar2=None,
                                                    op0=ALU.mult)
                        else:
                            wtmp = s2.tile([128, 256], F32, tag="wtmp", bufs=2)
                            nc.vector.tensor_scalar(out=wtmp[:], in0=vfn_tv[:],
                                                    scalar1=na_col, scalar2=None,
                                                    op0=ALU.mult)
                            nc.gpsimd.tensor_tensor(out=w[:], in0=wtmp[:],
                                                    in1=asf32(w_prev[:]),
                                                    op=ALU.add)
                        w_prev = w
                    wT = s2.tile([128, 256], F32R, tag="wT")
                    transpose256(p2t, s2, w_prev[:, 0:128], w_prev[:, 128:256],
                                 wT[:], "tp2")
                    wvp_ps = p2m.tile([128, 258], F32, tag="mm2")
                    mm_group(wvp_ps[:], [wT[:, 0:128], wT[:, 128:256]],
                             [vw_sb[:, 0:258], vw_sb[:, 258:516]],
                             (ones1_sb[:], vbr_sb[:]) if has_vb else None)
                    r4, nm4 = ln_stats(s2, wvp_ps, 256, "wvp")
                    wvp_t = s2.tile([128, 256], F32R, tag="wvpsb")
                    ln_apply(s2, wvp_t[:], wvp_ps[:, 0:256], r4, nm4,
                             "v" if has_va else None, True, "wvp")
                    wvpT = s2.tile([128, 256], F32R, tag="wvpT")
                    transpose256(p2t, s2, wvp_t[:, 0:128], wvp_t[:, 128:256],
                                 wvpT[:], "tp2")
                    fA = p2m.tile([128, 256], F32, tag="fa")
                    fB = p2m.tile([128, 258], F32, tag="fb")
                    lhs4 = [ppT_sb[t][:, 0:128], ppT_sb[t][:, 128:256],
                            wvpT[:, 0:128], wvpT[:, 128:256]]
                    mm_group(fA[:], lhs4,
                             [fw_sb[:, 514 * k:514 * k + 256] for k in range(4)],
                             (ones1_sb[:], fbr_sb[:, 0:256]) if has_fb else None)
                    mm_group(fB[:], lhs4,
                             [fw_sb[:, 514 * k + 256:514 * k + 514]
                              for k in range(4)],
                             (ones1_sb[:], fbr_sb[:, 256:514]) if has_fb else None)
                    # final LN over 512 (no relu, no interior affine: host applies f_g/f_be)
                    sqa = s2.tile([128, 256], F32, tag="fsqa")
                    sa = s2.tile([128, 1], F32, tag="fsa")
                    nc.scalar.activation(out=sqa[:], in_=fA[:], func=AF.Square,
                                         accum_out=sa[:])
                    sqb = s2.tile([128, 256], F32, tag="fsqb")
                    sb_ = s2.tile([128, 1], F32, tag="fsb")
                    nc.scalar.activation(out=sqb[:], in_=fB[:, 0:256],
                                         func=AF.Square, accum_out=sb_[:])
                    s2s = s2.tile([128, 1], F32, tag="fs2")
                    nc.vector.tensor_tensor(out=s2s[:], in0=sa[:], in1=sb_[:],
                                            op=ALU.add)
                    fm = s2.tile([128, 1], F32, tag="ffm")
                    nc.vector.tensor_scalar(out=fm[:], in0=fB[:, 256:257],
                                            scalar1=1.0 / 512, scalar2=None,
                                            op0=ALU.mult)
                    fmsq = s2.tile([128, 1], F32, tag="ffmsq")
                    nc.gpsimd.tensor_tensor(out=fmsq[:], in0=fm[:], in1=fm[:],
                                            op=ALU.mult)
                    fvar = s2.tile([128, 1], F32, tag="ffvar")
                    nc.vector.scalar_tensor_tensor(out=fvar[:], in0=s2s[:],
                                                   scalar=1.0 / 512, in1=fmsq[:],
                                                   op0=ALU.mult, op1=ALU.subtract)
                    fstd = s2.tile([128, 1], F32, tag="ffstd")
                    nc.scalar.activation(out=fstd[:], in_=fvar[:], func=AF.Sqrt,
                                         bias=eps_sb[:])
                    fr = s2.tile([128, 1], F32, tag="ffr")
                    nc.vector.reciprocal(fr[:], fstd[:])
                    fnegmr = s2.tile([128, 1], F32, tag="ffnegmr")
                    nc.vector.scalar_tensor_tensor(out=fnegmr[:], in0=fm[:],
                                                   scalar=-1.0, in1=fr[:],
                                                   op0=ALU.mult, op1=ALU.mult)
                    out_t = s2.tile([128, 512], F32, tag="outsb")
                    nc.scalar.activation(out=out_t[:, 0:256], in_=fA[:],
                                         func=AF.Identity, scale=fr[:],
                                         bias=fnegmr[:])
                    nc.scalar.activation(out=out_t[:, 256:512], in_=fB[:, 0:256],
                                         func=AF.Identity, scale=fr[:],
                                         bias=fnegmr[:])
                    nc.gpsimd.dma_start(outp_d[t], out_t[:])

    nc.compile()
    return nc


def _wchunks(W, n_in_chunks, mean_cols=True):
    # W: [K, N] -> [128, n*(N+2)] image: per chunk [block | rowsum | 0]
    K, N = W.shape
    z = np.zeros((128, 1), dtype=np.float32)
    cols = []
    for k in range(n_in_chunks):
        blk = W[128 * k:128 * (k + 1), :]
        cols.append(blk)
        cols.append(blk.sum(axis=1, keepdims=True))
        cols.append(z)
    return np.ascontiguousarray(np.concatenate(cols, axis=1), dtype=np.float32)


def kernel(**inputs):
    global LAST_RESULT
    pf = np.ascontiguousarray(inputs["point_features"], dtype=np.float32)
    vf = np.ascontiguousarray(inputs["view_features"], dtype=np.float32)
    sp = np.asarray(inputs["superpoint_ids"])
    p_W = np.asarray(inputs["p_W"], dtype=np.float32)
    v_W = np.asarray(inputs["v_W"], dtype=np.float32)
    a1_W = np.asarray(inputs["a1_W"], dtype=np.float32)
    a2_W = np.asarray(inputs["a2_W"], dtype=np.float32)
    f_W = np.asarray(inputs["f_W"], dtype=np.float32)
    p_b = np.asarray(inputs["p_b"], dtype=np.float32)
    v_b = np.asarray(inputs["v_b"], dtype=np.float32)
    a1_b = np.asarray(inputs["a1_b"], dtype=np.float32)
    a2_b = np.asarray(inputs["a2_b"], dtype=np.float32)
    f_b = np.asarray(inputs["f_b"], dtype=np.float32)
    p_g = np.asarray(inputs["p_g"], dtype=np.float32)
    v_g = np.asarray(inputs["v_g"], dtype=np.float32)
    a_g = np.asarray(inputs["a_g"], dtype=np.float32)
    f_g = np.asarray(inputs["f_g"], dtype=np.float32)
    p_be = np.asarray(inputs["p_be"], dtype=np.float32)
    v_be = np.asarray(inputs["v_be"], dtype=np.float32)
    a_be = np.asarray(inputs["a_be"], dtype=np.float32)
    f_be = np.asarray(inputs["f_be"], dtype=np.float32)

    has_pb = bool(np.any(p_b != 0))
    has_vb = bool(np.any(v_b != 0))
    has_a1b = bool(np.any(a1_b != 0))
    has_fb = bool(np.any(f_b != 0))
    has_pa = not (np.all(p_g == 1) and np.all(p_be == 0))
    has_va = not (np.all(v_g == 1) and np.all(v_be == 0))
    has_aa = not (np.all(a_g == 1) and np.all(a_be == 0))
    a2bias = float(a2_b.reshape(-1)[0])
    flags = (has_pb, has_vb, has_a1b, has_fb, has_pa, has_va, has_aa, a2bias)

    if flags not in _PROGRAM_CACHE:
        _PROGRAM_CACHE[flags] = _build_program(flags)
    nc = _PROGRAM_CACHE[flags]

    # shared weight images
    pw_img = _wchunks(p_W, 2)
    vw_img = _wchunks(v_W, 2)
    a1w_img = _wchunks(a1_W, 4)
    fw_cols = []
    for k in range(4):
        blk = f_W[128 * k:128 * (k + 1), :]
        fw_cols.append(blk[:, 0:256])
        fw_cols.append(blk[:, 256:512])
        fw_cols.append(blk.sum(axis=1, keepdims=True))
        fw_cols.append(np.zeros((128, 1), dtype=np.float32))
    fw_img = np.ascontiguousarray(np.concatenate(fw_cols, axis=1), dtype=np.float32)
    a2b_img = np.ascontiguousarray(
        np.tile(a2_W[:, 0][None, :], (128, 1)), dtype=np.float32)
    iota_img = np.ascontiguousarray(
        np.tile(np.arange(128, dtype=np.float32)[None, :], (128, 1)))
    eyer_img = np.ascontiguousarray(np.eye(128, dtype=np.float32))

    shared = {"pw": pw_img, "vw": vw_img, "a1w": a1w_img, "fw": fw_img,
              "a2b": a2b_img, "iota": iota_img, "eyer": eyer_img}
    if has_pb or has_vb or has_a1b or has_fb:
        shared["ones1"] = np.ones((1, 128), dtype=np.float32)
    if has_pb:
        shared["pbr"] = np.concatenate(
            [p_b, [p_b.sum(), 0.0]]).astype(np.float32)[None, :]
    if has_vb:
        shared["vbr"] = np.concatenate(
            [v_b, [v_b.sum(), 0.0]]).astype(np.float32)[None, :]
    if has_a1b:
        shared["a1br"] = np.concatenate(
            [a1_b, [a1_b.sum(), 0.0]]).astype(np.float32)[None, :]
    if has_fb:
        shared["fbr"] = np.concatenate(
            [f_b, [f_b.sum(), 0.0]]).astype(np.float32)[None, :]
    for key, flag, g, be in (("paff", has_pa, p_g, p_be),
                             ("vaff", has_va, v_g, v_be),
                             ("aaff", has_aa, a_g, a_be)):
        if flag:
            img = np.concatenate([np.tile(g[None, :], (128, 1)),
                                  np.tile(be[None, :], (128, 1))], axis=1)
            shared[key] = np.ascontiguousarray(img, dtype=np.float32)

    # per-core images
    # pft: [16,128,256] with [p, 128k+c] = pf[b, off+128t+c, 128k+p]
    pf5 = pf.reshape(B, 2, NT, 128, 2, 128)           # b, half, t, c, k, p
    pft_all = np.ascontiguousarray(
        pf5.transpose(0, 1, 2, 5, 4, 3).reshape(B, 2, NT, 128, 256))
    vf7 = vf.reshape(B, M, 2, NT, 128, 2, 128)        # b, v, half, t, c, k, p
    vft_all = np.ascontiguousarray(
        vf7.transpose(0, 2, 3, 1, 6, 5, 4).reshape(B, 2, NT, M, 128, 256))
    vfn_all = np.ascontiguousarray(
        vf.reshape(B, M, 2, NT, 128, 256).transpose(0, 2, 3, 1, 4, 5))
    spf_all = np.ascontiguousarray(
        sp.astype(np.float32).reshape(B, 2, NT, 128).transpose(0, 1, 3, 2))

    in_maps = []
    for c in range(8):
        b, h = c // 2, c % 2
        im = dict(shared)
        im["pft"] = pft_all[b, h]
        im["vft"] = vft_all[b, h]
        im["vfn"] = vfn_all[b, h]
        im["spf"] = spf_all[b, h]
        in_maps.append(im)

    res = run_bass_kernel_spmd(nc, in_maps, core_ids=list(range(8)),
                               trace=TRACE)
    LAST_RESULT = res

    out = np.empty((B, NPTS, FD), dtype=np.float32)
    for c in range(8):
        b, h = c // 2, c % 2
        out[b, h * HALF:(h + 1) * HALF, :] = \
            res.results[c]["outp"].reshape(HALF, FD)
    if not (np.all(f_g == 1) and np.all(f_be == 0)):
        out = out * f_g[None, None, :] + f_be[None, None, :]
    return out


# revision 66
# speedup vs baseline: 1.7644x; 1.7644x over previous
import sys

sys.path.insert(0, "/opt/trn_rl_repo")
import numpy as np

import concourse.mybir as mybir
import concourse.tile as tile
from concourse import bacc
from concourse.bass_utils import run_bass_kernel_spmd

F32 = mybir.dt.float32
F32R = mybir.dt.float32r
AF = mybir.ActivationFunctionType
ALU = mybir.AluOpType
AX = mybir.AxisListType

B, M, NPTS, D, FD, S = 4, 8, 4096, 256, 512, 128
NT = 16          # 128-point tiles per core
HALF = NT * 128  # 2048 points per core
EPS_LN = 1e-5
RG = [[0, 1], [2, 3], [4, 5], [6, 7]]

TRACE = False          # test.py may set kernel.TRACE = True
LAST_RESULT = None     # exposes BassKernelResults to test.py

_PROGRAM_CACHE = {}


def _build_program(flags):
    (has_pb, has_vb, has_a1b, has_fb, has_pa, has_va, has_aa, a2bias) = flags
    nc = bacc.Bacc("TRN2", target_bir_lowering=False, debug=False)

    pft_d = nc.dram_tensor("pft", [NT, 128, 256], F32R, kind="ExternalInput")
    vfn_d = nc.dram_tensor("vfn", [NT, M, 128, 256], F32R, kind="ExternalInput")
    spf_d = nc.dram_tensor("spf", [128, NT], F32, kind="ExternalInput")
    pw_d = nc.dram_tensor("pw", [128, 2 * 258], F32R, kind="ExternalInput")
    vw_d = nc.dram_tensor("vw", [128, 2 * 258], F32R, kind="ExternalInput")
    a1w_d = nc.dram_tensor("a1w", [128, 4 * 258], F32R, kind="ExternalInput")
    fw_d = nc.dram_tensor("fw", [128, 4 * 514], F32R, kind="ExternalInput")
    a2b_d = nc.dram_tensor("a2b", [128, 256], F32, kind="ExternalInput")
    iota_d = nc.dram_tensor("iota", [128, 128], F32, kind="ExternalInput")
    eyer_d = nc.dram_tensor("eyer", [128, 128], F32R, kind="ExternalInput")
    outp_d = nc.dram_tensor("outp", [NT, 128, 512], F32, kind="ExternalOutput")

    need_bias = has_pb or has_vb or has_a1b or has_fb
    if need_bias:
        ones1_d = nc.dram_tensor("ones1", [1, 128], F32R, kind="ExternalInput")
    if has_pb:
        pbr_d = nc.dram_tensor("pbr", [1, 258], F32R, kind="ExternalInput")
    if has_vb:
        vbr_d = nc.dram_tensor("vbr", [1, 258], F32R, kind="ExternalInput")
    if has_a1b:
        a1br_d = nc.dram_tensor("a1br", [1, 258], F32R, kind="ExternalInput")
    if has_fb:
        fbr_d = nc.dram_tensor("fbr", [1, 514], F32R, kind="ExternalInput")
    # interior gain/bias broadcast images (gain row, then be row), [128,512]
    if has_pa:
        paff_d = nc.dram_tensor("paff", [128, 512], F32, kind="ExternalInput")
    if has_va:
        vaff_d = nc.dram_tensor("vaff", [128, 512], F32, kind="ExternalInput")
    if has_aa:
        aaff_d = nc.dram_tensor("aaff", [128, 512], F32, kind="ExternalInput")

    with tile.TileContext(nc) as tc:
        with tc.tile_pool(name="wp", bufs=1) as wp, \
             tc.tile_pool(name="pers", bufs=1) as pers, \
             tc.tile_pool(name="dp", bufs=1, space="DRAM") as dp:
            # ---- persistent weights/constants ----
            pw_sb = wp.tile([128, 2 * 258], F32R, tag="pw")
            vw_sb = wp.tile([128, 2 * 258], F32R, tag="vw")
            a1w_sb = wp.tile([128, 4 * 258], F32R, tag="a1w")
            fw_sb = wp.tile([128, 4 * 514], F32R, tag="fw")
            a2b_sb = wp.tile([128, 256], F32, tag="a2b")
            iota_sb = wp.tile([128, 128], F32, tag="iota")
            eyer_sb = wp.tile([128, 128], F32R, tag="eyer")
            spf_sb = wp.tile([128, NT], F32, tag="spf")
            for sb, d in ((pw_sb, pw_d), (vw_sb, vw_d), (a1w_sb, a1w_d),
                          (fw_sb, fw_d), (a2b_sb, a2b_d), (iota_sb, iota_d),
                          (eyer_sb, eyer_d), (spf_sb, spf_d)):
                nc.gpsimd.dma_start(sb[:], d[:])
            if need_bias:
                ones1_sb = wp.tile([1, 128], F32R, tag="ones1")
                nc.gpsimd.dma_start(ones1_sb[:], ones1_d[:])
            if has_pb:
                pbr_sb = wp.tile([1, 258], F32R, tag="pbr")
                nc.gpsimd.dma_start(pbr_sb[:], pbr_d[:])
            if has_vb:
                vbr_sb = wp.tile([1, 258], F32R, tag="vbr")
                nc.gpsimd.dma_start(vbr_sb[:], vbr_d[:])
            if has_a1b:
                a1br_sb = wp.tile([1, 258], F32R, tag="a1br")
                nc.gpsimd.dma_start(a1br_sb[:], a1br_d[:])
            if has_fb:
                fbr_sb = wp.tile([1, 514], F32R, tag="fbr")
                nc.gpsimd.dma_start(fbr_sb[:], fbr_d[:])
            aff_sb = {}
            for key, flag, d in (("p", has_pa, "paff"), ("v", has_va, "vaff"),
                                 ("a", has_aa, "aaff")):
                if flag:
                    t_ = wp.tile([128, 512], F32, tag=d)
                    d_t = {"paff": paff_d, "vaff": vaff_d, "aaff": aaff_d}[d]
                    nc.gpsimd.dma_start(t_[:], d_t[:])
                    aff_sb[key] = t_

            # ---- persistent state ----
            pp_sb = [pers.tile([128, 266], F32R, tag=f"pp{t}", name=f"pp{t}") for t in range(NT)]
            ppT_sb = [pers.tile([128, 256], F32R, tag=f"ppT{t}", name=f"ppT{t}") for t in range(NT)]
            oh_sb = [pers.tile([128, 128], F32R, tag=f"oh{t}", name=f"oh{t}") for t in range(NT)]
            ohT_sb = [pers.tile([128, 128], F32R, tag=f"ohT{t}", name=f"ohT{t}") for t in range(NT)]
            nf2_sb = pers.tile([128, NT], F32, tag="nf2", name="nf2")
            apre_sb = pers.tile([128, 128], F32, tag="apre", name="apre")
            am_sb = pers.tile([128, 128], F32, tag="amall", name="amall")
            dots_sb = pers.tile([128, NT], F32, tag="dots", name="dots")
            prods_sb = pers.tile([128, NT], F32, tag="prods", name="prods")
            sims_sb = pers.tile([128, NT], F32, tag="sims", name="sims")
            ref_sb = pers.tile([128, 128], F32, tag="refined", name="refined")
            na_sb = pers.tile([128, 128], F32, tag="naall", name="naall")
            nmax_sb = pers.tile([128, NT], F32, tag="nmax", name="nmax")
            sume_sb = pers.tile([128, NT], F32, tag="sume", name="sume")
            rse_sb = pers.tile([128, NT], F32, tag="rse", name="rse")
            segp_sb = pers.tile([128, 266], F32, tag="segpre", name="segpre")
            seg_sb = pers.tile([128, 266], F32, tag="segsb", name="segsb")
            mean_sb = pers.tile([128, 266], F32R, tag="meansb", name="meansb")
            eps_sb = pers.tile([128, 1], F32, tag="epsln", name="epsln")
            nc.gpsimd.memset(eps_sb[:], EPS_LN)
            a2bv_sb = pers.tile([128, 1], F32, tag="a2bv", name="a2bv")
            nc.gpsimd.memset(a2bv_sb[:], a2bias)
            onecol_sb = pers.tile([128, 2], F32, tag="onecol", name="onecol")
            nc.gpsimd.memset(onecol_sb[:], 1.0)

            bounce_in = dp.tile([128, 266], F32, tag="bin")
            bounce_out = dp.tile([128, 266], F32, tag="bout")

            def asf32(ap):
                return ap.bitcast(F32) if ap.dtype == F32R else ap

            def mm_group(out_ap, lhs_list, rhs_list, bias_pair=None):
                n = len(lhs_list) + (1 if bias_pair is not None else 0)
                i = 0
                for l, r in zip(lhs_list, rhs_list):
                    nc.tensor.matmul(out_ap, l, r,
                                     start=(i == 0), stop=(i == n - 1))
                    i += 1
                if bias_pair is not None:
                    nc.tensor.matmul(out_ap, bias_pair[0], bias_pair[1],
                                     start=(i == 0), stop=True)

            def ln_stats(pool, ps, width, tagp):
                # ps: psum [128, width+1] with mean-sum col at `width`.
                # returns (r, negmr) [128,1] APs
                sq = pool.tile([128, width], F32, tag=tagp + "sq")
                s2 = pool.tile([128, 1], F32, tag=tagp + "s2")
                nc.scalar.activation(out=sq[:], in_=ps[:, 0:width],
                                     func=AF.Square, accum_out=s2[:])
                m = pool.tile([128, 1], F32, tag=tagp + "m")
                nc.vector.tensor_scalar(out=m[:], in0=ps[:, width:width + 1],
                                        scalar1=1.0 / width, scalar2=None,
                                        op0=ALU.mult)
                msq = pool.tile([128, 1], F32, tag=tagp + "msq")
                nc.gpsimd.tensor_tensor(out=msq[:], in0=m[:], in1=m[:], op=ALU.mult)
                var = pool.tile([128, 1], F32, tag=tagp + "var")
                nc.vector.scalar_tensor_tensor(out=var[:], in0=s2[:],
                                               scalar=1.0 / width, in1=msq[:],
                                               op0=ALU.mult, op1=ALU.subtract)
                std = pool.tile([128, 1], F32, tag=tagp + "std")
                nc.scalar.activation(out=std[:], in_=var[:], func=AF.Sqrt,
                                     bias=eps_sb[:])
                r = pool.tile([128, 1], F32, tag=tagp + "r")
                nc.vector.reciprocal(r[:], std[:])
                negmr = pool.tile([128, 1], F32, tag=tagp + "negmr")
                nc.vector.scalar_tensor_tensor(out=negmr[:], in0=m[:], scalar=-1.0,
                                               in1=r[:], op0=ALU.mult, op1=ALU.mult)
                return r, negmr

            def ln_apply(pool, out_ap, ps_ap, r, negmr, aff_key, relu, tagp):
                # out = [relu]( LN(x)*g + be ); unit-affine fast path uses one ACT op
                if aff_key is None:
                    nc.scalar.activation(out=out_ap, in_=ps_ap,
                                         func=(AF.Relu if relu else AF.Identity),
                                         scale=r[:], bias=negmr[:])
                    return
                g_ap = aff_sb[aff_key][:, 0:256]
                be_ap = aff_sb[aff_key][:, 256:512]
                t1 = pool.tile([128, 256], F32, tag=tagp + "t1")
                nc.scalar.activation(out=t1[:], in_=ps_ap, func=AF.Identity,
                                     scale=r[:], bias=negmr[:])
                t2 = pool.tile([128, 256], F32, tag=tagp + "t2")
                nc.vector.tensor_tensor(out=t2[:], in0=t1[:], in1=g_ap, op=ALU.mult)
                if relu:
                    t3 = pool.tile([128, 256], F32, tag=tagp + "t3")
                    nc.vector.scalar_tensor_tensor(out=t3[:], in0=t2[:], scalar=0.0,
                                                   in1=be_ap, op0=ALU.bypass,
                                                   op1=ALU.add)
                    nc.vector.tensor_scalar(out=out_ap, in0=t3[:], scalar1=0.0,
                                            scalar2=None, op0=ALU.max)
                else:
                    nc.vector.scalar_tensor_tensor(out=out_ap, in0=t2[:], scalar=0.0,
                                                   in1=be_ap, op0=ALU.bypass,
                                                   op1=ALU.add)

            def transpose256(pool_ps, pool_sb, src_sb_ap0, src_sb_ap1, dst_sb_ap,
                             tagp):
                tp = pool_ps.tile([128, 256], F32R, tag=tagp)
                nc.tensor.transpose(tp[:, 0:128], src_sb_ap0, eyer_sb[:])
                nc.tensor.transpose(tp[:, 128:256], src_sb_ap1, eyer_sb[:])
                nc.scalar.activation(out=dst_sb_ap, in_=asf32(tp[:]),
                                     func=AF.Copy)

            # ================= phase 1 =================
            with tc.tile_pool(name="s1", bufs=2) as s1, \
                 tc.tile_pool(name="p1", bufs=2, space="PSUM") as p1, \
                 tc.tile_pool(name="pg", bufs=1, space="PSUM") as pg:
                for t in range(NT):
                    pft_t = s1.tile([128, 256], F32R, tag="pft")
                    nc.gpsimd.dma_start(pft_t[:], pft_d[t])
                    pp_ps = p1.tile([128, 258], F32, tag="mm")
                    mm_group(pp_ps[:], [pft_t[:, 0:128], pft_t[:, 128:256]],
                             [pw_sb[:, 0:258], pw_sb[:, 258:516]],
                             (ones1_sb[:], pbr_sb[:]) if has_pb else None)
                    r0, nm0 = ln_stats(s1, pp_ps, 256, "pp")
                    ln_apply(s1, pp_sb[t][:, 0:256], pp_ps[:, 0:256], r0, nm0,
                             "p" if has_pa else None, True, "pp")
                    # |f|^2 per point
                    sqn = s1.tile([128, 256], F32, tag="sqn")
                    nc.vector.scalar_tensor_tensor(
                        out=sqn[:], in0=asf32(pp_sb[t][:, 0:256]), scalar=0.0,
                        in1=asf32(pp_sb[t][:, 0:256]), op0=ALU.bypass,
                        op1=ALU.mult, accum_out=nf2_sb[:, t:t + 1])
                    # one-hot segment matrix
                    nc.vector.tensor_scalar(out=oh_sb[t][:], in0=iota_sb[:],
                                            scalar1=spf_sb[:, t:t + 1],
                                            scalar2=None, op0=ALU.is_equal)
                    nc.scalar.activation(out=pp_sb[t][:, 264:266],
                                         in_=onecol_sb[:], func=AF.Copy)
                    transpose256(p1, s1, pp_sb[t][:, 0:128], pp_sb[t][:, 128:256],
                                 ppT_sb[t][:], "tp")

                    for v in range(M):
                        vft_tv = s1.tile([128, 256], F32R, tag="vft", bufs=3)
                        nc.gpsimd.dma_start(vft_tv[:], vfn_d[t, v])
                        vfT_t = s1.tile([128, 256], F32R, tag="vfT")
                        transpose256(p1, s1, vft_tv[:, 0:128],
                                     vft_tv[:, 128:256], vfT_t[:], "tp")
                        vp_ps = p1.tile([128, 258], F32, tag="mm")
                        mm_group(vp_ps[:], [vfT_t[:, 0:128], vfT_t[:, 128:256]],
                                 [vw_sb[:, 0:258], vw_sb[:, 258:516]],
                                 (ones1_sb[:], vbr_sb[:]) if has_vb else None)
                        r1, nm1 = ln_stats(s1, vp_ps, 256, "vp")
                        vp_t = s1.tile([128, 256], F32R, tag="vpsb")
                        ln_apply(s1, vp_t[:], vp_ps[:, 0:256], r1, nm1,
                                 "v" if has_va else None, True, "vp")
                        vpT_t = s1.tile([128, 256], F32R, tag="vpT")
                        transpose256(p1, s1, vp_t[:, 0:128], vp_t[:, 128:256],
                                     vpT_t[:], "tp")
                        h_ps = p1.tile([128, 258], F32, tag="hmm")
                        mm_group(h_ps[:],
                                 [ppT_sb[t][:, 0:128], ppT_sb[t][:, 128:256],
                                  vpT_t[:, 0:128], vpT_t[:, 128:256]],
                                 [a1w_sb[:, 0:258], a1w_sb[:, 258:516],
                                  a1w_sb[:, 516:774], a1w_sb[:, 774:1032]],
                                 (ones1_sb[:], a1br_sb[:]) if has_a1b else None)
                        r2, nm2 = ln_stats(s1, h_ps, 256, "h")
                        h_t = s1.tile([128, 256], F32, tag="hsb")
                        ln_apply(s1, h_t[:], h_ps[:, 0:256], r2, nm2,
                                 "a" if has_aa else None, True, "h")
                        sqd = s1.tile([128, 256], F32, tag="sqd")
                        nc.vector.scalar_tensor_tensor(
                            out=sqd[:], in0=h_t[:], scalar=0.0, in1=a2b_sb[:],
                            op0=ALU.bypass, op1=ALU.mult,
                            accum_out=apre_sb[:, 8 * t + v:8 * t + v + 1])

                # ---- phase 1b: sigmoid (one ACT table switch) ----
                for t in range(NT):
                    nc.scalar.activation(out=pp_sb[t][:, 256:264],
                                         in_=apre_sb[:, 8 * t:8 * t + 8],
                                         func=AF.Sigmoid, bias=a2bv_sb[:])

                # ---- phase 1c: segment sums + AllReduce ----
                seg_ps = pg.tile([128, 266], F32, tag="seg")
                for t in range(NT):
                    nc.tensor.matmul(seg_ps[:], oh_sb[t][:], pp_sb[t][:, 0:266],
                                     start=(t == 0), stop=(t == NT - 1))
                nc.scalar.activation(out=segp_sb[:], in_=seg_ps[:], func=AF.Copy)
                nc.gpsimd.dma_start(bounce_in[:], segp_sb[:])
                nc.gpsimd.collective_compute(
                    "AllReduce", ALU.add, replica_groups=RG,
                    ins=[bounce_in.opt()], outs=[bounce_out.opt()])
                nc.gpsimd.dma_start(seg_sb[:], bounce_out[:])

            # ================= phase 2 =================
            with tc.tile_pool(name="s2", bufs=2) as s2, \
                 tc.tile_pool(name="p2t", bufs=2, space="PSUM") as p2t, \
                 tc.tile_pool(name="p2g", bufs=2, space="PSUM") as p2g, \
                 tc.tile_pool(name="p2m", bufs=1, space="PSUM") as p2m:
                # segment means + |fm|^2
                cntc = s2.tile([128, 1], F32, tag="cntc")
                nc.vector.tensor_scalar(out=cntc[:], in0=seg_sb[:, 264:265],
                                        scalar1=1.0, scalar2=None, op0=ALU.max)
                rc = s2.tile([128, 1], F32, tag="rc")
                nc.vector.reciprocal(rc[:], cntc[:])
                nc.vector.tensor_scalar(out=mean_sb[:, 0:264], in0=seg_sb[:, 0:264],
                                        scalar1=rc[:], scalar2=None, op0=ALU.mult)
                sqm = s2.tile([128, 256], F32, tag="sqm")
                m2tmp = s2.tile([128, 1], F32, tag="m2tmp")
                nc.vector.scalar_tensor_tensor(
                    out=sqm[:], in0=asf32(mean_sb[:, 0:256]), scalar=0.0,
                    in1=asf32(mean_sb[:, 0:256]), op0=ALU.bypass, op1=ALU.mult,
                    accum_out=m2tmp[:])
                nc.scalar.activation(out=mean_sb[:, 264:265], in_=m2tmp[:],
                                     func=AF.Copy)
                nc.scalar.activation(out=mean_sb[:, 265:266], in_=m2tmp[:],
                                     func=AF.Copy)

                # ---- phase 2a: gather + cosine ----
                for t in range(NT):
                    tpo = p2t.tile([128, 256], F32R, tag="tp2")
                    nc.tensor.transpose(tpo[:, 0:128], oh_sb[t][:], eyer_sb[:])
                    nc.scalar.activation(out=ohT_sb[t][:],
                                         in_=asf32(tpo[:, 0:128]), func=AF.Copy)
                    gath = p2g.tile([128, 266], F32, tag="gath")
                    nc.tensor.matmul(gath[:], ohT_sb[t][:], mean_sb[:, 0:266],
                                     start=True, stop=True)
                    sqg = s2.tile([128, 256], F32, tag="sqg")
                    nc.vector.scalar_tensor_tensor(
                        out=sqg[:], in0=asf32(pp_sb[t][:, 0:256]), scalar=0.0,
                        in1=gath[:, 0:256], op0=ALU.bypass, op1=ALU.mult,
                        accum_out=dots_sb[:, t:t + 1])
                    nc.vector.tensor_tensor(out=prods_sb[:, t:t + 1],
                                            in0=nf2_sb[:, t:t + 1],
                                            in1=gath[:, 264:265], op=ALU.mult)
                    nc.scalar.activation(out=am_sb[:, 8 * t:8 * t + 8],
                                         in_=gath[:, 256:264], func=AF.Copy)

                dens = s2.tile([128, NT], F32, tag="dens")
                nc.scalar.activation(out=dens[:], in_=prods_sb[:], func=AF.Sqrt)
                densc = s2.tile([128, NT], F32, tag="densc")
                nc.vector.tensor_scalar(out=densc[:], in0=dens[:], scalar1=1e-16,
                                        scalar2=None, op0=ALU.max)
                rden = s2.tile([128, NT], F32, tag="rden")
                nc.vector.reciprocal(rden[:], densc[:])
                nc.vector.tensor_tensor(out=sims_sb[:], in0=dots_sb[:], in1=rden[:],
                                        op=ALU.mult)
                for t in range(NT):
                    dtl = s2.tile([128, 8], F32, tag="dtl")
                    nc.gpsimd.tensor_tensor(out=dtl[:],
                                            in0=asf32(pp_sb[t][:, 256:264]),
                                            in1=am_sb[:, 8 * t:8 * t + 8],
                                            op=ALU.subtract)
                    nc.vector.scalar_tensor_tensor(
                        out=ref_sb[:, 8 * t:8 * t + 8], in0=dtl[:],
                        scalar=sims_sb[:, t:t + 1],
                        in1=am_sb[:, 8 * t:8 * t + 8], op0=ALU.mult, op1=ALU.add)

                # ---- phase 2b: softmax over views (exp table switch) ----
                for t in range(NT):
                    nc.vector.tensor_reduce(out=nmax_sb[:, t:t + 1],
                                            in_=ref_sb[:, 8 * t:8 * t + 8],
                                            axis=AX.X, op=ALU.max, negate=True)
                for t in range(NT):
                    nc.scalar.activation(out=na_sb[:, 8 * t:8 * t + 8],
                                         in_=ref_sb[:, 8 * t:8 * t + 8],
                                         func=AF.Exp,
                                         bias=nmax_sb[:, t:t + 1],
                                         accum_out=sume_sb[:, t:t + 1])
                nc.vector.reciprocal(rse_sb[:], sume_sb[:])
                for t in range(NT):
                    nc.vector.tensor_scalar(out=na_sb[:, 8 * t:8 * t + 8],
                                            in0=na_sb[:, 8 * t:8 * t + 8],
                                            scalar1=rse_sb[:, t:t + 1],
                                            scalar2=None, op0=ALU.mult)

                # ---- phase 2c: weighted sum, wvp, final ----
                for t in range(NT):
                    w_prev = None
                    for v in range(M):
                        vfn_tv = s2.tile([128, 256], F32R, tag="vfn", bufs=4)
                        nc.gpsimd.dma_start(vfn_tv[:], vfn_d[t, v])
                        na_col = na_sb[:, 8 * t + v:8 * t + v + 1]
                        w = s2.tile([128, 256], F32R, tag="wacc", bufs=2)
                        if v == 0:
                            nc.vector.tensor_scalar(out=w[:], in0=asf32(vfn_tv[:]),
                                                    scalar1=na_col, scalar2=None,
                                                    op0=ALU.mult)
                        else:
                            wtmp = s2.tile([128, 256], F32, tag="wtmp", bufs=2)
                            nc.vector.tensor_scalar(out=wtmp[:], in0=asf32(vfn_tv[:]),
                                                    scalar1=na_col, scalar2=None,
                                                    op0=ALU.mult)
                            nc.gpsimd.tensor_tensor(out=w[:], in0=wtmp[:],
                                                    in1=asf32(w_prev[:]),
                                                    op=ALU.add)
                        w_prev = w
                    wT = s2.tile([128, 256], F32R, tag="wT")
                    transpose256(p2t, s2, w_prev[:, 0:128], w_prev[:, 128:256],
                                 wT[:], "tp2")
                    wvp_ps = p2m.tile([128, 258], F32, tag="mm2")
                    mm_group(wvp_ps[:], [wT[:, 0:128], wT[:, 128:256]],
                             [vw_sb[:, 0:258], vw_sb[:, 258:516]],
                             (ones1_sb[:], vbr_sb[:]) if has_vb else None)
                    r4, nm4 = ln_stats(s2, wvp_ps, 256, "wvp")
                    wvp_t = s2.tile([128, 256], F32R, tag="wvpsb")
                    ln_apply(s2, wvp_t[:], wvp_ps[:, 0:256], r4, nm4,
                             "v" if has_va else None, True, "wvp")
                    wvpT = s2.tile([128, 256], F32R, tag="wvpT")
                    transpose256(p2t, s2, wvp_t[:, 0:128], wvp_t[:, 128:256],
                                 wvpT[:], "tp2")
                    fA = p2m.tile([128, 256], F32, tag="fa")
                    fB = p2m.tile([128, 258], F32, tag="fb")
                    lhs4 = [ppT_sb[t][:, 0:128], ppT_sb[t][:, 128:256],
                            wvpT[:, 0:128], wvpT[:, 128:256]]
                    mm_group(fA[:], lhs4,
                             [fw_sb[:, 514 * k:514 * k + 256] for k in range(4)],
                             (ones1_sb[:], fbr_sb[:, 0:256]) if has_fb else None)
                    mm_group(fB[:], lhs4,
                             [fw_sb[:, 514 * k + 256:514 * k + 514]
                              for k in range(4)],
                             (ones1_sb[:], fbr_sb[:, 256:514]) if has_fb else None)
                    # final LN over 512 (no relu, no interior affine: host applies f_g/f_be)
                    sqa = s2.tile([128, 256], F32, tag="fsqa")
                    sa = s2.tile([128, 1], F32, tag="fsa")
                    nc.scalar.activation(out=sqa[:], in_=fA[:], func=AF.Square,
                                         accum_out=sa[:])
                    sqb = s2.tile([128, 256], F32, tag="fsqb")
                    sb_ = s2.tile([128, 1], F32, tag="fsb")
                    nc.scalar.activation(out=sqb[:], in_=fB[:, 0:256],
                                         func=AF.Square, accum_out=sb_[:])
                    s2s = s2.tile([128, 1], F32, tag="fs2")
                    nc.vector.tensor_tensor(out=s2s[:], in0=sa[:], in1=sb_[:],
                                            op=ALU.add)
                    fm = s2.tile([128, 1], F32, tag="ffm")
                    nc.vector.tensor_scalar(out=fm[:], in0=fB[:, 256:257],
                                            scalar1=1.0 / 512, scalar2=None,
                                            op0=ALU.mult)
                    fmsq = s2.tile([128, 1], F32, tag="ffmsq")
                    nc.gpsimd.tensor_tensor(out=fmsq[:], in0=fm[:], in1=fm[:],
                                            op=ALU.mult)
                    fvar = s2.tile([128, 1], F32, tag="ffvar")
                    nc.vector.scalar_tensor_tensor(out=fvar[:], in0=s2s[:],
                                                   scalar=1.0 / 512, in1=fmsq[:],
                                                   op0=ALU.mult, op1=ALU.subtract)
                    fstd = s2.tile([128, 1], F32, tag="ffstd")
                    nc.scalar.activation(out=fstd[:], in_=fvar[:], func=AF.Sqrt,
                                         bias=eps_sb[:])
                    fr = s2.tile([128, 1], F32, tag="ffr")
                    nc.vector.reciprocal(fr[:], fstd[:])
                    fnegmr = s2.tile([128, 1], F32, tag="ffnegmr")
                    nc.vector.scalar_tensor_tensor(out=fnegmr[:], in0=fm[:],
                                                   scalar=-1.0, in1=fr[:],
                                                   op0=ALU.mult, op1=ALU.mult)
                    out_t = s2.tile([128, 512], F32, tag="outsb")
                    nc.scalar.activation(out=out_t[:, 0:256], in_=fA[:],
                                         func=AF.Identity, scale=fr[:],
                                         bias=fnegmr[:])
                    nc.scalar.activation(out=out_t[:, 256:512], in_=fB[:, 0:256],
                                         func=AF.Identity, scale=fr[:],
                                         bias=fnegmr[:])
                    nc.gpsimd.dma_start(outp_d[t], out_t[:])

    nc.compile()
    return nc


def _wchunks(W, n_in_chunks, mean_cols=True):
    # W: [K, N] -> [128, n*(N+2)] image: per chunk [block | rowsum | 0]
    K, N = W.shape
    z = np.zeros((128, 1), dtype=np.float32)
    cols = []
    for k in range(n_in_chunks):
        blk = W[128 * k:128 * (k + 1), :]
        cols.append(blk)
        cols.append(blk.sum(axis=1, keepdims=True))
        cols.append(z)
    return np.ascontiguousarray(np.concatenate(cols, axis=1), dtype=np.float32)


def kernel(**inputs):
    global LAST_RESULT
    pf = np.ascontiguousarray(inputs["point_features"], dtype=np.float32)
    vf = np.ascontiguousarray(inputs["view_features"], dtype=np.float32)
    sp = np.asarray(inputs["superpoint_ids"])
    p_W = np.asarray(inputs["p_W"], dtype=np.float32)
    v_W = np.asarray(inputs["v_W"], dtype=np.float32)
    a1_W = np.asarray(inputs["a1_W"], dtype=np.float32)
    a2_W = np.asarray(inputs["a2_W"], dtype=np.float32)
    f_W = np.asarray(inputs["f_W"], dtype=np.float32)
    p_b = np.asarray(inputs["p_b"], dtype=np.float32)
    v_b = np.asarray(inputs["v_b"], dtype=np.float32)
    a1_b = np.asarray(inputs["a1_b"], dtype=np.float32)
    a2_b = np.asarray(inputs["a2_b"], dtype=np.float32)
    f_b = np.asarray(inputs["f_b"], dtype=np.float32)
    p_g = np.asarray(inputs["p_g"], dtype=np.float32)
    v_g = np.asarray(inputs["v_g"], dtype=np.float32)
    a_g = np.asarray(inputs["a_g"], dtype=np.float32)
    f_g = np.asarray(inputs["f_g"], dtype=np.float32)
    p_be = np.asarray(inputs["p_be"], dtype=np.float32)
    v_be = np.asarray(inputs["v_be"], dtype=np.float32)
    a_be = np.asarray(inputs["a_be"], dtype=np.float32)
    f_be = np.asarray(inputs["f_be"], dtype=np.float32)

    has_pb = bool(np.any(p_b != 0))
    has_vb = bool(np.any(v_b != 0))
    has_a1b = bool(np.any(a1_b != 0))
    has_fb = bool(np.any(f_b != 0))
    has_pa = not (np.all(p_g == 1) and np.all(p_be == 0))
    has_va = not (np.all(v_g == 1) and np.all(v_be == 0))
    has_aa = not (np.all(a_g == 1) and np.all(a_be == 0))
    a2bias = float(a2_b.reshape(-1)[0])
    flags = (has_pb, has_vb, has_a1b, has_fb, has_pa, has_va, has_aa, a2bias)

    if flags not in _PROGRAM_CACHE:
        _PROGRAM_CACHE[flags] = _build_program(flags)
    nc = _PROGRAM_CACHE[flags]

    # shared weight images
    pw_img = _wchunks(p_W, 2)
    vw_img = _wchunks(v_W, 2)
    a1w_img = _wchunks(a1_W, 4)
    fw_cols = []
    for k in range(4):
        blk = f_W[128 * k:128 * (k + 1), :]
        fw_cols.append(blk[:, 0:256])
        fw_cols.append(blk[:, 256:512])
        fw_cols.append(blk.sum(axis=1, keepdims=True))
        fw_cols.append(np.zeros((128, 1), dtype=np.float32))
    fw_img = np.ascontiguousarray(np.concatenate(fw_cols, axis=1), dtype=np.float32)
    a2b_img = np.ascontiguousarray(
        np.tile(a2_W[:, 0][None, :], (128, 1)), dtype=np.float32)
    iota_img = np.ascontiguousarray(
        np.tile(np.arange(128, dtype=np.float32)[None, :], (128, 1)))
    eyer_img = np.ascontiguousarray(np.eye(128, dtype=np.float32))

    shared = {"pw": pw_img, "vw": vw_img, "a1w": a1w_img, "fw": fw_img,
              "a2b": a2b_img, "iota": iota_img, "eyer": eyer_img}
    if has_pb or has_vb or has_a1b or has_fb:
        shared["ones1"] = np.ones((1, 128), dtype=np.float32)
    if has_pb:
        shared["pbr"] = np.concatenate(
            [p_b, [p_b.sum(), 0.0]]).astype(np.float32)[None, :]
    if has_vb:
        shared["vbr"] = np.concatenate(
            [v_b, [v_b.sum(), 0.0]]).astype(np.float32)[None, :]
    if has_a1b:
        shared["a1br"] = np.concatenate(
            [a1_b, [a1_b.sum(), 0.0]]).astype(np.float32)[None, :]
    if has_fb:
        shared["fbr"] = np.concatenate(
            [f_b, [f_b.sum(), 0.0]]).astype(np.float32)[None, :]
    for key, flag, g, be in (("paff", has_pa, p_g, p_be),
                             ("vaff", has_va, v_g, v_be),
                             ("aaff", has_aa, a_g, a_be)):
        if flag:
            img = np.concatenate([np.tile(g[None, :], (128, 1)),
                                  np.tile(be[None, :], (128, 1))], axis=1)
            shared[key] = np.ascontiguousarray(img, dtype=np.float32)

    # per-core images
    # pft: [16,128,256] with [p, 128k+c] = pf[b, off+128t+c, 128k+p]
    pf5 = pf.reshape(B, 2, NT, 128, 2, 128)           # b, half, t, c, k, p
    pft_all = np.ascontiguousarray(
        pf5.transpose(0, 1, 2, 5, 4, 3).reshape(B, 2, NT, 128, 256))
    vfn_all = np.ascontiguousarray(
        vf.reshape(B, M, 2, NT, 128, 256).transpose(0, 2, 3, 1, 4, 5))
    spf_all = np.ascontiguousarray(
        sp.astype(np.float32).reshape(B, 2, NT, 128).transpose(0, 1, 3, 2))

    in_maps = []
    for c in range(8):
        b, h = c // 2, c % 2
        im = dict(shared)
        im["pft"] = pft_all[b, h]
        im["vfn"] = vfn_all[b, h]
        im["spf"] = spf_all[b, h]
        in_maps.append(im)

    res = run_bass_kernel_spmd(nc, in_maps, core_ids=list(range(8)),
                               trace=TRACE)
    LAST_RESULT = res

    out = np.empty((B, NPTS, FD), dtype=np.float32)
    for c in range(8):
        b, h = c // 2, c % 2
        out[b, h * HALF:(h + 1) * HALF, :] = \
            res.results[c]["outp"].reshape(HALF, FD)
    if not (np.all(f_g == 1) and np.all(f_be == 0)):
        out = out * f_g[None, None, :] + f_be[None, None, :]
    return out
